# revision 28
# baseline (speedup 1.0000x reference)
"""Trainium2 Bass kernel: CausalCrossAttention (GroupNorm + Q proj + block-causal
cross-attention over a small context + out proj + residual), 8-core SPMD.

Sharding: each of the 8 cores owns one (batch b, frame-residue r) pair:
  b = core // 4, r = core % 4, frames t = r + 4*f for f in 0..3.
All per-frame work is core-local (k/v come from the tiny per-batch context).

Design (v3) vs the f32 baseline (114-128us):
  * All DMA I/O is bf16 (x, out, weights cast host-side): ~10MB/core HBM
    traffic instead of 22MB, both HWDGE rings streaming from t=0 (x0 first,
    then k-side weights on sync; params + v-side on scalar).
  * The kv projection runs in fp8 (ctx, wkv*64 host-cast) with DoubleRow
    matmuls: 2048 PE cycles instead of 8192; the 1/64 descale rides the
    PSUM->SBUF evacuation scale for free.
  * GroupNorm is folded into the attention algebra: h = a*x+b per channel
    means scores = (a.*kq)^T x + (kq^T b)[s] -- a tiny per-frame rescale of
    the fused kq = Wq^T k matrix plus a per-s bias column, so no normalize
    pass over [512, 1024] ever runs and the PE consumes the DMA'd x directly.
  * Softmax in the [s, q] layout with zero transposes: one ACT Exp with the
    causal mask + score bias as the per-partition activation bias, denominator
    broadcast via a ones-matmul, DVE fast-reciprocal, p*linv on GpSimd.
    Only {Exp, Identity, Copy} activation funcs -> a single ACT table set
    (rsqrt for the norm is a quake-style seed + 1 Newton step on DVE).
  * Residual via PE identity-matmul accumulation into the out-proj PSUM; ACT
    evacuates PSUM into the x tile (bf16), which is the out-DMA source.
  * Stats: 8x bn_stats (HW FMAX 512) on DVE per frame, even/odd merge on
    GpSimd, group fold/expand via tiny matmuls (halves folded by accumulating
    two strided-rhs matmuls).
  * 2-deep software pipeline: iteration f emits finish(f) -> scores(f) ->
    Exp(f) -> out(f-1)+evac+DMA interleaved with l(f) -> bn(f+1) -> linv(f)
    -> pn(f) -> merge(f+1), so DVE runs [quake_f, bn_{f+1}, linv_f] with no
    FIFO stalls and the PE never waits on ACT.
"""

import numpy as np
import ml_dtypes

import concourse.bass as bass
import concourse.bacc as bacc
import concourse.mybir as mybir
import concourse.tile as tile
from concourse.bass_utils import run_bass_kernel_spmd
from concourse.masks import make_identity

# Problem shape (fixed by the harness).
B, C, T, H, W = 2, 512, 16, 32, 32
HW = H * W            # 1024 query positions per frame
S, D = 64, 1024       # context length, context dim
G = 32                # groupnorm groups
CPG = C // G          # 16 channels per group
NCORES = 8
FPC = (B * T) // NCORES   # 4 frames per core
NCH = C // 128        # 4 channel chunks of 128
NDCH = D // 128       # 8 context-dim chunks
EPS = 1e-5
SCALE = float(C) ** -0.5
NEGINF = -1e9
# quake rsqrt seed magic, pre-adjusted for taking bits of 0.5*x instead of x
MAGIC_HALF = 0x5F3759DF - 0x00400000
W8SCALE = 64.0        # fp8 pre-scale for wkv (values ~N(0, 1.28^2) in e4m3)

F32 = mybir.dt.float32
BF16 = mybir.dt.bfloat16
FP8 = mybir.dt.float8e4
I32 = mybir.dt.int32
NP_BF16 = ml_dtypes.bfloat16
NP_FP8 = ml_dtypes.float8_e4m3

Identity = mybir.ActivationFunctionType.Identity
Copy = mybir.ActivationFunctionType.Copy
Exp = mybir.ActivationFunctionType.Exp
Alu = mybir.AluOpType
DR = mybir.MatmulPerfMode.DoubleRow

# prm column layout: [gammaT 0:4 | betaT 4:8 | gmat/64 8:16 | maskcols 16:20]
PRM_W = 20

LAST_RESULT = None        # BassKernelResults of the most recent run (for test.py)
_GRAPH_CACHE = {}


def _build(with_bq: bool, with_bkv: bool, with_bo: bool) -> bass.Bass:
    nc = bacc.Bacc()

    x_d = nc.declare_dram_parameter("x", [128, FPC, NCH, HW], BF16, isOutput=False)
    xf8_d = nc.declare_dram_parameter("xf8", [128, FPC, NCH, HW], FP8, isOutput=False)
    ctx_d = nc.declare_dram_parameter("ctxT_pm", [128, NDCH, S], FP8, isOutput=False)
    wq_d = nc.declare_dram_parameter("wq_pm", [128, NCH, C], FP8, isOutput=False)
    wkvk_d = nc.declare_dram_parameter("wkvk_pm", [128, NDCH, C], FP8, isOutput=False)
    wkvv_d = nc.declare_dram_parameter("wkvv_pm", [128, NDCH, C], FP8, isOutput=False)
    wo_d = nc.declare_dram_parameter("wo_pm", [128, NCH, C], BF16, isOutput=False)
    prm_d = nc.declare_dram_parameter("prm", [128, PRM_W], F32, isOutput=False)
    emat_d = nc.declare_dram_parameter("emat", [8, 128], F32, isOutput=False)
    if with_bq:
        bq_d = nc.declare_dram_parameter("bqT", [128, NCH], F32, isOutput=False)
    if with_bkv:
        bkv_d = nc.declare_dram_parameter("bkv", [1, 2 * C], F32, isOutput=False)
    if with_bo:
        bo_d = nc.declare_dram_parameter("bo", [1, C], F32, isOutput=False)
    out_d = nc.declare_dram_parameter("out", [128, FPC, NCH, HW], BF16, isOutput=True)

    with tile.TileContext(nc) as tc:
        with (
            tc.tile_pool(name="wp", bufs=1) as wp,
            tc.tile_pool(name="xp", bufs=4) as xp,
            tc.tile_pool(name="fr", bufs=2) as fr,
            tc.tile_pool(name="sm", bufs=2) as sm,
            tc.tile_pool(name="psA", bufs=1, space="PSUM") as psA,
            tc.tile_pool(name="psO", bufs=2, space="PSUM") as psO,
            tc.tile_pool(name="psT", bufs=2, space="PSUM") as psT,
        ):
            # ---------------- DMA streams (both HWDGE rings start at t=0) ----
            wq_f8 = wp.tile([128, NCH, C], FP8)
            wkvk_f8 = wp.tile([128, NDCH, C], FP8)
            wkvv_f8 = wp.tile([128, NDCH, C], FP8)
            wo_bf = wp.tile([128, NCH, C], BF16)
            ctx_f8 = wp.tile([128, NDCH, S], FP8)
            prm = wp.tile([128, PRM_W], F32)
            emat_sb = wp.tile([8, 128], F32)

            x_tiles = [xp.tile([128, NCH, HW], BF16, name="x_sb", tag="x_sb")
                       for _ in range(FPC)]
            x8_tiles = [xp.tile([128, NCH, HW], FP8, name="x8_sb", tag="x8_sb")
                        for _ in range(FPC)]
            # x0 in quarters so frame-0 bn_stats starts ASAP; weights follow.
            for ci in range(NCH):
                nc.sync.dma_start(out=x_tiles[0][:, ci:ci + 1, :],
                                  in_=x_d[:, 0, ci:ci + 1, :])
            nc.sync.dma_start(out=wkvk_f8[:], in_=wkvk_d[:, :, :])
            nc.sync.dma_start(out=ctx_f8[:], in_=ctx_d[:, :, :])
            nc.sync.dma_start(out=wq_f8[:], in_=wq_d[:, :, :])
            nc.sync.dma_start(out=x8_tiles[0][:], in_=xf8_d[:, 0, :, :])
            for f in range(1, FPC):
                nc.sync.dma_start(out=x_tiles[f][:], in_=x_d[:, f, :, :])
                nc.sync.dma_start(out=x8_tiles[f][:], in_=xf8_d[:, f, :, :])

            nc.scalar.dma_start(out=prm[:], in_=prm_d[:, :])
            nc.scalar.dma_start(out=emat_sb[:], in_=emat_d[:, :])
            nc.scalar.dma_start(out=wkvv_f8[:], in_=wkvv_d[:, :, :])
            nc.scalar.dma_start(out=wo_bf[:], in_=wo_d[:, :, :])
            if with_bq:
                bqT_sb = wp.tile([128, NCH], F32)
                nc.scalar.dma_start(out=bqT_sb[:], in_=bq_d[:, :])
            if with_bkv:
                bkv_sb = wp.tile([1, 2 * C], F32)
                nc.scalar.dma_start(out=bkv_sb[:], in_=bkv_d[:, :])
            if with_bo:
                bo_sb = wp.tile([1, C], F32)
                nc.scalar.dma_start(out=bo_sb[:], in_=bo_d[:, :])

            # ---------------- small constants --------------------------------
            identity = wp.tile([128, 128], BF16)
            ones64 = wp.tile([64, 64], BF16)
            c256 = wp.tile([128, 1], F32)
            ci256 = wp.tile([8, 1], F32)
            chalf = wp.tile([8, 1], F32)
            cepsh = wp.tile([8, 1], F32)
            c8 = wp.tile([128, 1], F32)
            magic_sb = wp.tile([8, NCH], I32)
            make_identity(nc, identity[:])
            nc.vector.memset(ones64[:], 1.0)
            nc.vector.memset(c256[:], 256.0)
            nc.vector.memset(ci256[:], 0.5 / 256.0)   # E2fold/256 then *0.5
            nc.vector.memset(chalf[:], 0.5)
            nc.vector.memset(cepsh[:], 0.5 * EPS)
            nc.vector.memset(c8[:], 8.0)
            nc.gpsimd.memset(magic_sb[:], MAGIC_HALF)
            if with_bkv or with_bo:
                ones1s = wp.tile([1, S], BF16)
                nc.vector.memset(ones1s[:], 1.0)

            if with_bkv:
                bkv_bf = wp.tile([1, 2 * C], BF16)
                nc.gpsimd.tensor_copy(out=bkv_bf[:], in_=bkv_sb[:])
            if with_bo:
                bo_bf = wp.tile([1, C], BF16)
                nc.gpsimd.tensor_copy(out=bo_bf[:], in_=bo_sb[:])

            # ---------------- per-frame statistics ---------------------------
            st2_tiles = [None] * FPC
            st6_tiles = [None] * FPC

            def emit_stats_bn(f):
                # DVE: 8x bn_stats over 512-blocks of the bf16 x tile
                x_sb = x_tiles[f]
                xv = x_sb[:].rearrange("p a (b w) -> p (a b) w", b=2)
                st6 = fr.tile([128, 8, 6], F32, tag="st6")
                for j in range(8):
                    nc.vector.bn_stats(out=st6[:, j, :], in_=xv[:, j, :])
                st6_tiles[f] = st6
                return st6

            def emit_stats_merge(f, st6):
                # GpSimd: merge even/odd streams ->
                #   st2[.,.,0] = mean_e + mean_o (= 2*mean_block)
                #   st2[.,.,1] = (M2_e + M2_o) + 256*(mean_e^2 + mean_o^2)
                st2 = fr.tile([128, 8, 2], F32, tag="st2")
                nc.gpsimd.tensor_add(st2[:, :, 0], st6[:, :, 1], st6[:, :, 4])
                nc.gpsimd.tensor_mul(st6[:, :, 0], st6[:, :, 1], st6[:, :, 1])
                nc.gpsimd.tensor_mul(st6[:, :, 3], st6[:, :, 4], st6[:, :, 4])
                nc.gpsimd.tensor_add(st6[:, :, 0], st6[:, :, 0], st6[:, :, 3])
                nc.gpsimd.tensor_add(st6[:, :, 2], st6[:, :, 2], st6[:, :, 5])
                nc.gpsimd.tensor_mul(st6[:, :, 0], st6[:, :, 0],
                                     c256[:].to_broadcast((128, 8)))
                nc.gpsimd.tensor_add(st2[:, :, 1], st6[:, :, 0], st6[:, :, 2])
                # pre-merge the two 512-halves -> [128, ci, 2]
                st2m = fr.tile([128, NCH, 2], F32, tag="st2m")
                st2v = st2[:].rearrange("p (a b) c -> p a b c", b=2)
                nc.gpsimd.tensor_add(st2m[:], st2v[:, :, 0, :], st2v[:, :, 1, :])
                st2_tiles[f] = st2m

            def emit_finish_fold(f):
                # fold over partitions+halves: 2 accumulating matmuls with
                # strided rhs -> psum_g[band j, (ci, kind)] (gmat scaled 1/64)
                ps_g = psT.tile([8, NCH, 2], F32, tag="pst")
                nc.tensor.matmul(
                    ps_g[:], lhsT=prm[:, 8:16], rhs=st2_tiles[f][:],
                    start=True, stop=True)
                gsb = fr.tile([8, NCH, 2], F32, tag="gsb")
                nc.scalar.activation(out=gsb[:], in_=ps_g[:], func=Copy)
                return gsb

            def emit_finish_hx(gsb):
                # hx = 0.5*(var + eps) = gsb1*(0.5/256) - 0.5*mu^2 + 0.5*eps
                msq = fr.tile([8, NCH], F32, tag="msq")
                nc.gpsimd.tensor_mul(msq[:], gsb[:, :, 0], gsb[:, :, 0])
                nc.gpsimd.tensor_mul(msq[:], msq[:],
                                     chalf[:].to_broadcast((8, NCH)))
                hx = fr.tile([8, NCH], F32, tag="hx")
                nc.gpsimd.tensor_mul(hx[:], gsb[:, :, 1],
                                     ci256[:].to_broadcast((8, NCH)))
                nc.gpsimd.tensor_sub(hx[:], hx[:], msq[:])
                nc.gpsimd.tensor_add(hx[:], hx[:],
                                     cepsh[:].to_broadcast((8, NCH)))
                return hx

            def emit_finish_quake(gsb, hx):
                # quake rsqrt, one positive-form Newton step (DVE)
                sh = fr.tile([8, NCH], I32, tag="sh")
                nc.vector.tensor_scalar(
                    out=sh[:], in0=hx[:].bitcast(I32), scalar1=1, scalar2=None,
                    op0=Alu.arith_shift_right)
                ya = fr.tile([8, NCH], F32, tag="ya")
                nc.vector.tensor_sub(ya[:].bitcast(I32), magic_sb[:], sh[:])
                u = fr.tile([8, NCH], F32, tag="u")
                nc.vector.tensor_mul(u[:], ya[:], ya[:])
                nc.vector.tensor_mul(u[:], u[:], hx[:])
                nc.vector.tensor_mul(u[:], u[:], ya[:])
                nc.vector.scalar_tensor_tensor(
                    out=gsb[:, :, 1], in0=ya[:], scalar=1.5, in1=u[:],
                    op0=Alu.mult, op1=Alu.subtract)   # istd = 1.5*ya - ya*u

            def emit_finish_expand(gsb):
                # expand to channels: psum_e[c, (ci, 2)] = emat^T @ gsb
                ps_e = psT.tile([128, NCH, 2], F32, tag="pst")
                nc.tensor.matmul(
                    ps_e[:].rearrange("p a b -> p (a b)"), lhsT=emat_sb[:],
                    rhs=gsb[:].rearrange("p a b -> p (a b)"),
                    start=True, stop=True)
                mi = fr.tile([128, NCH, 2], F32, tag="mi")
                nc.scalar.activation(out=mi[:], in_=ps_e[:], func=Copy)
                return mi

            # ---------------- context constants: k/v, kq, vo -----------------
            kT_f8 = wp.tile([128, NCH, S], FP8)
            vT_sb = wp.tile([128, NCH, S], BF16)

            emit_stats_bn(0)

            for half in range(2):
                wsrc = wkvk_f8 if half == 0 else wkvv_f8
                ps_kv = psT.tile([S, C], F32, tag="pst")
                for i in range(NDCH // 2):
                    nc.tensor.matmul(
                        ps_kv[:], lhsT=ctx_f8[:, 2 * i:2 * i + 2, :],
                        rhs=wsrc[:, 2 * i:2 * i + 2, :],
                        start=(i == 0),
                        stop=(i == NDCH // 2 - 1 and not with_bkv),
                        perf_mode=DR)
                if with_bkv:
                    nc.tensor.matmul(
                        ps_kv[:], lhsT=ones1s[:],
                        rhs=bkv_bf[:, half * C:(half + 1) * C],
                        start=False, stop=True)
                kv_sb = sm.tile([S, C], BF16, tag="kv")
                nc.scalar.activation(out=kv_sb[:], in_=ps_kv[:], func=Copy,
                                     scale=1.0 / W8SCALE)
                ps_t = psT.tile([128, NCH, S], BF16, tag="pst")
                for ci in range(NCH):
                    nc.tensor.transpose(
                        ps_t[:, ci, :], kv_sb[:, ci * 128:(ci + 1) * 128],
                        identity[:64, :64])
                dst = kT_f8 if half == 0 else vT_sb
                nc.scalar.activation(out=dst[:], in_=ps_t[:], func=Copy)


            # kq[c, s] = sum_o wq[o, c] k[s, o]  (f32 kept for per-frame scale)
            kq_sb = wp.tile([128, NCH, S], F32)
            ps_kq = psT.tile([128, NCH, S], F32, tag="pst")
            for co in range(NCH):
                for i in range(NCH // 2):
                    nc.tensor.matmul(
                        ps_kq[:, co, :],
                        lhsT=wq_f8[:, 2 * i:2 * i + 2, co * 128:(co + 1) * 128],
                        rhs=kT_f8[:, 2 * i:2 * i + 2, :],
                        start=(i == 0), stop=(i == NCH // 2 - 1),
                        perf_mode=DR)
            nc.scalar.activation(out=kq_sb[:], in_=ps_kq[:], func=Copy,
                                 scale=1.0 / W8SCALE)
            kq_bf = wp.tile([128, NCH, S], BF16)
            nc.gpsimd.tensor_copy(out=kq_bf[:], in_=kq_sb[:])

            # vo[s, oc] = sum_c v[s, c] wo[oc, c]  (+ bo row: softmax sums to 1)
            vo_bf = wp.tile([S, C], BF16)
            ps_vo = psT.tile([S, C], F32, tag="pst")
            for ci in range(NCH):
                nc.tensor.matmul(
                    ps_vo[:], lhsT=vT_sb[:, ci, :], rhs=wo_bf[:, ci, :],
                    start=(ci == 0), stop=(ci == NCH - 1 and not with_bo))
            if with_bo:
                nc.tensor.matmul(
                    ps_vo[:], lhsT=ones1s[:], rhs=bo_bf[:],
                    start=False, stop=True)
            nc.scalar.activation(out=vo_bf[:], in_=ps_vo[:], func=Copy)

            # bqk[s] = sum_o bq[o] k[s, o] -> folded into all mask columns
            if with_bq:
                bq_bf = wp.tile([128, NCH], FP8)
                nc.gpsimd.tensor_copy(out=bq_bf[:], in_=bqT_sb[:])
                ps_bq = psT.tile([S, 1], F32, tag="pst")
                for ci in range(NCH):
                    nc.tensor.matmul(
                        ps_bq[:], lhsT=kT_f8[:, ci, :], rhs=bq_bf[:, ci:ci + 1],
                        start=(ci == 0), stop=(ci == NCH - 1))
                nc.vector.scalar_tensor_tensor(
                    out=prm[:S, 16:20], in0=ps_bq[:].to_broadcast((S, 4)),
                    scalar=SCALE, in1=prm[:S, 16:20],
                    op0=Alu.mult, op1=Alu.add)

            # ---------------- 2-deep pipelined frame loop --------------------
            # Per-engine FIFO orders are chosen so no engine head-blocks:
            #   DVE : quake(f), bn(f+1), evac-oc2/3(f-1), linv(f)
            #   GPS : merge(f), hx(f), ab/kqf(f), pn(f)
            #   PE  : fold(f), out(f-1) oc0/1, expand(f), bias(f), scores(f),
            #         out oc2, l(f), out oc3
            #   ACT : gsb(f), mi(f), biascol(f), evac-oc0/1(f-1), Exp(f)
            pending = [None]

            def emit_out_mms(ent, oc, preadd):
                bf_, bpn, bx = ent
                ps_o = psO.tile([128, 2, 512], F32, tag="ps_o")
                for hf in range(2):
                    nc.tensor.matmul(
                        ps_o[:, hf, :],
                        lhsT=vo_bf[:, oc * 128:(oc + 1) * 128],
                        rhs=bpn[:, hf, :], start=True, stop=not preadd)
                    if preadd:
                        nc.tensor.matmul(
                            ps_o[:, hf, :], lhsT=identity[:],
                            rhs=bx[:, oc, hf * 512:(hf + 1) * 512],
                            start=False, stop=True)
                return ps_o

            for f in range(FPC):
                x_sb = x_tiles[f]
                ps_sc = psA.tile([S, 2, 512], F32, tag="ps_sc")
                ent = pending[0]
                pending[0] = None

                emit_stats_merge(f, st6_tiles[f])
                gsb = emit_finish_fold(f)

                ps_o01 = []
                if ent is not None:
                    ps_o01.append(emit_out_mms(ent, 0, preadd=True))
                    ps_o01.append(emit_out_mms(ent, 1, preadd=True))

                hx = emit_finish_hx(gsb)
                emit_finish_quake(gsb, hx)
                mi = emit_finish_expand(gsb)

                # a = istd*gamma ; b = beta - mu*a ; kqf = a .* kq (GpSimd)
                ab = fr.tile([128, NCH, 2], F32, tag="ab")
                nc.gpsimd.tensor_mul(ab[:, :, 0], mi[:, :, 1], prm[:, 0:4])
                nc.gpsimd.tensor_mul(ab[:, :, 1], mi[:, :, 0], ab[:, :, 0])
                nc.gpsimd.tensor_sub(ab[:, :, 1], prm[:, 4:8], ab[:, :, 1])
                a8 = fr.tile([128, NCH, 1], F32, tag="a8")
                nc.gpsimd.tensor_mul(a8[:], ab[:, :, 0:1],
                                     c8[:, None, :].to_broadcast((128, NCH, 1)))
                kqf = fr.tile([128, NCH, S], FP8, tag="kqf")
                nc.gpsimd.tensor_mul(
                    kqf[:], kq_sb[:], a8[:].to_broadcast((128, NCH, S)))

                b_bf = fr.tile([128, NCH, 1], BF16, tag="b_bf")
                nc.gpsimd.tensor_copy(out=b_bf[:], in_=ab[:, :, 1:2])
                ps_b = psT.tile([S, 1], F32, tag="pst")
                for ci in range(NCH):
                    nc.tensor.matmul(
                        ps_b[:], lhsT=kq_bf[:, ci, :], rhs=b_bf[:, ci, :],
                        start=(ci == 0), stop=(ci == NCH - 1))
                biascol = fr.tile([S, 1], F32, tag="biascol")
                nc.scalar.activation(
                    out=biascol[:], in_=ps_b[:], func=Identity,
                    bias=prm[:S, 16 + f:17 + f], scale=SCALE)

                # ACT evacs of f-1 oc0/1 fill the gap before Exp(f)
                if ent is not None:
                    bx = ent[2]
                    for oc in range(2):
                        nc.scalar.activation(
                            out=bx[:, oc, :],
                            in_=ps_o01[oc][:].rearrange("p a b -> p (a b)"),
                            func=Copy)

                # scoresT[s, q] via fp8 DoubleRow (kqf holds 8*a*kq);
                # p = exp((SCALE/8)*scores8 + bias)
                x8 = x8_tiles[f]
                for hf in range(2):
                    for i in range(NCH // 2):
                        nc.tensor.matmul(
                            ps_sc[:, hf, :],
                            lhsT=kqf[:, 2 * i:2 * i + 2, :],
                            rhs=x8[:, 2 * i:2 * i + 2,
                                   hf * 512:(hf + 1) * 512],
                            start=(i == 0), stop=(i == NCH // 2 - 1),
                            perf_mode=DR)
                p_bf = fr.tile([S, 2, 512], BF16, tag="p_bf")
                nc.scalar.activation(
                    out=p_bf[:], in_=ps_sc[:], func=Exp,
                    bias=biascol[:], scale=SCALE / 8.0)

                # out(f-1) oc2 | l(f) | out(f-1) oc3 on the PE
                ps_o23 = []
                if ent is not None:
                    ps_o23.append(emit_out_mms(ent, 2, preadd=False))
                for hf in range(2):
                    nc.tensor.matmul(
                        ps_sc[:, hf, :], lhsT=ones64[:], rhs=p_bf[:, hf, :],
                        start=True, stop=True)
                if ent is not None:
                    ps_o23.append(emit_out_mms(ent, 3, preadd=False))

                # next frame's bn_stats ahead of the DVE evacs + linv
                if f + 1 < FPC:
                    emit_stats_bn(f + 1)

                if ent is not None:
                    bf_, bpn, bx = ent
                    for i, oc in enumerate((2, 3)):
                        nc.vector.tensor_tensor(
                            out=bx[:, oc, :],
                            in0=ps_o23[i][:].rearrange("p a b -> p (a b)"),
                            in1=bx[:, oc, :], op=Alu.add)
                    nc.scalar.dma_start(out=out_d[:, bf_, :, :], in_=bx[:])

                linv = fr.tile([S, 2, 512], F32, tag="linv")
                nc.vector.reciprocal_approx_fast(out=linv[:], in_=ps_sc[:])
                pn_bf = fr.tile([S, 2, 512], BF16, tag="pn_bf")
                nc.gpsimd.tensor_mul(pn_bf[:], p_bf[:], linv[:])

                pending[0] = (f, pn_bf, x_sb)

            # final frame flush: ACT evac + per-chunk DMA for earliest drain
            bf_, bpn, bx = pending[0]
            for oc in range(NCH):
                ps_o = emit_out_mms(pending[0], oc, preadd=True)
                nc.scalar.activation(
                    out=bx[:, oc, :],
                    in_=ps_o[:].rearrange("p a b -> p (a b)"), func=Copy)
                nc.scalar.dma_start(out=out_d[:, bf_, oc:oc + 1, :],
                                    in_=bx[:, oc:oc + 1, :])

    nc.finalize()
    return nc


def _prep_in_maps(x, context, gamma, beta, wq, bq, wkv, bkv, wo, bo):
    f32 = lambda a: np.asarray(a, dtype=np.float32)
    bf16c = lambda a: np.ascontiguousarray(a).astype(NP_BF16)
    fp8c = lambda a: np.ascontiguousarray(a).astype(NP_FP8)
    pm = lambda a, n: a.reshape(n, 128, a.shape[-1]).transpose(1, 0, 2)

    wq_c = fp8c(pm(f32(wq) * W8SCALE, NCH))               # [128, 4, C]
    wkvT = f32(wkv).T * W8SCALE                           # [D, 2C]
    wkvk_c = fp8c(pm(np.ascontiguousarray(wkvT[:, :C]), NDCH))
    wkvv_c = fp8c(pm(np.ascontiguousarray(wkvT[:, C:]), NDCH))
    woT_c = bf16c(pm(np.ascontiguousarray(f32(wo).T), NCH))

    prm_base = np.zeros((128, PRM_W), np.float32)
    prm_base[:, 0:4] = f32(gamma).reshape(NCH, 128).T
    prm_base[:, 4:8] = f32(beta).reshape(NCH, 128).T
    pidx = np.arange(128)
    prm_base[pidx, 8 + pidx // CPG] = 1.0 / 64.0

    emat = np.zeros((8, 128), np.float32)
    emat[pidx // CPG, pidx] = 1.0

    bqT_c = np.ascontiguousarray(f32(bq).reshape(NCH, 128).T)
    # kv PSUM carries W8SCALE*k (fp8 weight pre-scale); bias must match
    bkv_c = np.ascontiguousarray(f32(bkv).reshape(1, 2 * C)) * W8SCALE
    bo_r = np.ascontiguousarray(f32(bo).reshape(1, C))

    x_f = f32(x)
    ctx_f = f32(context)

    in_maps = []
    for core in range(NCORES):
        b, r = divmod(core, 4)
        xs_raw = np.ascontiguousarray(
            x_f[b, :, r::4, :, :].reshape(NCH, 128, FPC, HW).transpose(1, 2, 0, 3))
        xs = xs_raw.astype(NP_BF16)
        xs8 = xs_raw.astype(NP_FP8)
        ctxT = fp8c(pm(np.ascontiguousarray(ctx_f[b].T), NDCH))  # [128, 8, S]
        prm = prm_base.copy()
        for f in range(FPC):
            t = 4 * f + r
            lim = min(4 * (t + 1), S)
            prm[lim:S, 16 + f] = NEGINF
        m = dict(x=xs, xf8=xs8, ctxT_pm=ctxT, wq_pm=wq_c, wkvk_pm=wkvk_c,
                 wkvv_pm=wkvv_c, wo_pm=woT_c, prm=prm, emat=emat)
        if np.any(bqT_c):
            m["bqT"] = bqT_c
        if np.any(bkv_c):
            m["bkv"] = bkv_c
        if np.any(bo_r):
            m["bo"] = bo_r
        in_maps.append(m)
    return in_maps


def kernel(x, context, gamma, beta, wq, bq, wkv, bkv, wo, bo,
           _trace=False, **_trace_kwargs):
    global LAST_RESULT
    with_bq = bool(np.any(np.asarray(bq)))
    with_bkv = bool(np.any(np.asarray(bkv)))
    with_bo = bool(np.any(np.asarray(bo)))
    key = (with_bq, with_bkv, with_bo)
    if key not in _GRAPH_CACHE:
        _GRAPH_CACHE[key] = _build(*key)
    nc = _GRAPH_CACHE[key]

    in_maps = _prep_in_maps(x, context, gamma, beta, wq, bq, wkv, bkv, wo, bo)
    res = run_bass_kernel_spmd(nc, in_maps, core_ids=list(range(NCORES)),
                               trace=_trace, **_trace_kwargs)
    LAST_RESULT = res

    out = np.empty((B, C, T, H, W), np.float32)
    for core in range(NCORES):
        b, r = divmod(core, 4)
        arr = np.asarray(res.results[core]["out"], dtype=np.float32)
        out[b, :, r::4, :, :] = arr.transpose(2, 0, 1, 3).reshape(C, FPC, H, W)
    return out


# revision 29
# speedup vs baseline: 1.1127x; 1.1127x over previous
"""Trainium2 Bass kernel: CausalCrossAttention (GroupNorm + Q proj + block-causal
cross-attention over a small context + out proj + residual), 8-core SPMD.

Sharding: each of the 8 cores owns one (batch b, frame-residue r) pair:
  b = core // 4, r = core % 4, frames t = r + 4*f for f in 0..3.
All per-frame work is core-local (k/v come from the tiny per-batch context).

Design (v3) vs the f32 baseline (114-128us):
  * All DMA I/O is bf16 (x, out, weights cast host-side): ~10MB/core HBM
    traffic instead of 22MB, both HWDGE rings streaming from t=0 (x0 first,
    then k-side weights on sync; params + v-side on scalar).
  * The kv projection runs in fp8 (ctx, wkv*64 host-cast) with DoubleRow
    matmuls: 2048 PE cycles instead of 8192; the 1/64 descale rides the
    PSUM->SBUF evacuation scale for free.
  * GroupNorm is folded into the attention algebra: h = a*x+b per channel
    means scores = (a.*kq)^T x + (kq^T b)[s] -- a tiny per-frame rescale of
    the fused kq = Wq^T k matrix plus a per-s bias column, so no normalize
    pass over [512, 1024] ever runs and the PE consumes the DMA'd x directly.
  * Softmax in the [s, q] layout with zero transposes: one ACT Exp with the
    causal mask + score bias as the per-partition activation bias, denominator
    broadcast via a ones-matmul, DVE fast-reciprocal, p*linv on GpSimd.
    Only {Exp, Identity, Copy} activation funcs -> a single ACT table set
    (rsqrt for the norm is a quake-style seed + 1 Newton step on DVE).
  * Residual via PE identity-matmul accumulation into the out-proj PSUM; ACT
    evacuates PSUM into the x tile (bf16), which is the out-DMA source.
  * Stats: 8x bn_stats (HW FMAX 512) on DVE per frame, even/odd merge on
    GpSimd, group fold/expand via tiny matmuls (halves folded by accumulating
    two strided-rhs matmuls).
  * 2-deep software pipeline: iteration f emits finish(f) -> scores(f) ->
    Exp(f) -> out(f-1)+evac+DMA interleaved with l(f) -> bn(f+1) -> linv(f)
    -> pn(f) -> merge(f+1), so DVE runs [quake_f, bn_{f+1}, linv_f] with no
    FIFO stalls and the PE never waits on ACT.
"""

import numpy as np
import ml_dtypes

import concourse.bass as bass
import concourse.bacc as bacc
import concourse.mybir as mybir
import concourse.tile as tile
from concourse.bass_utils import run_bass_kernel_spmd
from concourse.masks import make_identity

# Problem shape (fixed by the harness).
B, C, T, H, W = 2, 512, 16, 32, 32
HW = H * W            # 1024 query positions per frame
S, D = 64, 1024       # context length, context dim
G = 32                # groupnorm groups
CPG = C // G          # 16 channels per group
NCORES = 8
FPC = (B * T) // NCORES   # 4 frames per core
NCH = C // 128        # 4 channel chunks of 128
NDCH = D // 128       # 8 context-dim chunks
EPS = 1e-5
SCALE = float(C) ** -0.5
NEGINF = -1e9
# quake rsqrt seed magic, pre-adjusted for taking bits of 0.5*x instead of x
MAGIC_HALF = 0x5F3759DF - 0x00400000
W8SCALE = 64.0        # fp8 pre-scale for wkv (values ~N(0, 1.28^2) in e4m3)

F32 = mybir.dt.float32
BF16 = mybir.dt.bfloat16
FP8 = mybir.dt.float8e4
I32 = mybir.dt.int32
NP_BF16 = ml_dtypes.bfloat16
NP_FP8 = ml_dtypes.float8_e4m3

Identity = mybir.ActivationFunctionType.Identity
Copy = mybir.ActivationFunctionType.Copy
Exp = mybir.ActivationFunctionType.Exp
Alu = mybir.AluOpType
DR = mybir.MatmulPerfMode.DoubleRow

# prm column layout: [gammaT 0:4 | betaT 4:8 | gmat/64 8:16 | maskcols 16:20]
PRM_W = 20

LAST_RESULT = None        # BassKernelResults of the most recent run (for test.py)
_GRAPH_CACHE = {}


def _build(with_bq: bool, with_bkv: bool, with_bo: bool) -> bass.Bass:
    nc = bacc.Bacc()

    x_d = nc.declare_dram_parameter("x", [128, FPC, NCH, HW], BF16, isOutput=False)
    ctx_d = nc.declare_dram_parameter("ctxT_pm", [128, NDCH, S], FP8, isOutput=False)
    wq_d = nc.declare_dram_parameter("wq_pm", [128, NCH, C], FP8, isOutput=False)
    wkvk_d = nc.declare_dram_parameter("wkvk_pm", [128, NDCH, C], FP8, isOutput=False)
    wkvv_d = nc.declare_dram_parameter("wkvv_pm", [128, NDCH, C], FP8, isOutput=False)
    wo_d = nc.declare_dram_parameter("wo_pm", [128, NCH, C], BF16, isOutput=False)
    prm_d = nc.declare_dram_parameter("prm", [128, PRM_W], F32, isOutput=False)
    emat_d = nc.declare_dram_parameter("emat", [8, 128], F32, isOutput=False)
    if with_bq:
        bq_d = nc.declare_dram_parameter("bqT", [128, NCH], F32, isOutput=False)
    if with_bkv:
        bkv_d = nc.declare_dram_parameter("bkv", [1, 2 * C], F32, isOutput=False)
    if with_bo:
        bo_d = nc.declare_dram_parameter("bo", [1, C], F32, isOutput=False)
    out_d = nc.declare_dram_parameter("out", [128, FPC, NCH, HW], BF16, isOutput=True)

    with tile.TileContext(nc) as tc:
        with (
            tc.tile_pool(name="wp", bufs=1) as wp,
            tc.tile_pool(name="xp", bufs=4) as xp,
            tc.tile_pool(name="fr", bufs=2) as fr,
            tc.tile_pool(name="sm", bufs=2) as sm,
            tc.tile_pool(name="psA", bufs=1, space="PSUM") as psA,
            tc.tile_pool(name="psO", bufs=2, space="PSUM") as psO,
            tc.tile_pool(name="psT", bufs=2, space="PSUM") as psT,
        ):
            # ---------------- DMA streams (both HWDGE rings start at t=0) ----
            wq_f8 = wp.tile([128, NCH, C], FP8)
            wkvk_f8 = wp.tile([128, NDCH, C], FP8)
            wkvv_f8 = wp.tile([128, NDCH, C], FP8)
            wo_bf = wp.tile([128, NCH, C], BF16)
            ctx_f8 = wp.tile([128, NDCH, S], FP8)
            prm = wp.tile([128, PRM_W], F32)
            emat_sb = wp.tile([8, 128], F32)

            x_tiles = [xp.tile([128, NCH, HW], BF16, name="x_sb", tag="x_sb")
                       for _ in range(FPC)]
            # x0 in quarters so frame-0 bn_stats starts ASAP; weights follow.
            for ci in range(NCH):
                nc.sync.dma_start(out=x_tiles[0][:, ci:ci + 1, :],
                                  in_=x_d[:, 0, ci:ci + 1, :])
            nc.sync.dma_start(out=wkvk_f8[:], in_=wkvk_d[:, :, :])
            nc.sync.dma_start(out=ctx_f8[:], in_=ctx_d[:, :, :])
            nc.sync.dma_start(out=wq_f8[:], in_=wq_d[:, :, :])
            for f in range(1, FPC):
                nc.sync.dma_start(out=x_tiles[f][:], in_=x_d[:, f, :, :])

            nc.scalar.dma_start(out=prm[:], in_=prm_d[:, :])
            nc.scalar.dma_start(out=emat_sb[:], in_=emat_d[:, :])
            nc.scalar.dma_start(out=wkvv_f8[:], in_=wkvv_d[:, :, :])
            nc.scalar.dma_start(out=wo_bf[:], in_=wo_d[:, :, :])
            if with_bq:
                bqT_sb = wp.tile([128, NCH], F32)
                nc.scalar.dma_start(out=bqT_sb[:], in_=bq_d[:, :])
            if with_bkv:
                bkv_sb = wp.tile([1, 2 * C], F32)
                nc.scalar.dma_start(out=bkv_sb[:], in_=bkv_d[:, :])
            if with_bo:
                bo_sb = wp.tile([1, C], F32)
                nc.scalar.dma_start(out=bo_sb[:], in_=bo_d[:, :])

            # ---------------- small constants --------------------------------
            identity = wp.tile([128, 128], BF16)
            ones64 = wp.tile([64, 64], BF16)
            c256 = wp.tile([128, 1], F32)
            ci256 = wp.tile([8, 1], F32)
            chalf = wp.tile([8, 1], F32)
            cepsh = wp.tile([8, 1], F32)
            magic_sb = wp.tile([8, NCH], I32)
            make_identity(nc, identity[:])
            nc.vector.memset(ones64[:], 1.0)
            nc.vector.memset(c256[:], 256.0)
            nc.vector.memset(ci256[:], 0.5 / 256.0)   # E2fold/256 then *0.5
            nc.vector.memset(chalf[:], 0.5)
            nc.vector.memset(cepsh[:], 0.5 * EPS)
            nc.gpsimd.memset(magic_sb[:], MAGIC_HALF)
            if with_bkv or with_bo:
                ones1s = wp.tile([1, S], BF16)
                nc.vector.memset(ones1s[:], 1.0)

            if with_bkv:
                bkv_bf = wp.tile([1, 2 * C], BF16)
                nc.gpsimd.tensor_copy(out=bkv_bf[:], in_=bkv_sb[:])
            if with_bo:
                bo_bf = wp.tile([1, C], BF16)
                nc.gpsimd.tensor_copy(out=bo_bf[:], in_=bo_sb[:])

            # ---------------- per-frame statistics ---------------------------
            st2_tiles = [None] * FPC
            st6_tiles = [None] * FPC

            def emit_stats_bn(f):
                # DVE: 8x bn_stats over 512-blocks of the bf16 x tile
                x_sb = x_tiles[f]
                xv = x_sb[:].rearrange("p a (b w) -> p (a b) w", b=2)
                st6 = fr.tile([128, 8, 6], F32, tag="st6")
                for j in range(8):
                    nc.vector.bn_stats(out=st6[:, j, :], in_=xv[:, j, :])
                st6_tiles[f] = st6
                return st6

            def emit_stats_merge(f, st6):
                # GpSimd: merge even/odd streams ->
                #   st2[.,.,0] = mean_e + mean_o (= 2*mean_block)
                #   st2[.,.,1] = (M2_e + M2_o) + 256*(mean_e^2 + mean_o^2)
                st2 = fr.tile([128, 8, 2], F32, tag="st2")
                nc.gpsimd.tensor_add(st2[:, :, 0], st6[:, :, 1], st6[:, :, 4])
                nc.gpsimd.tensor_mul(st6[:, :, 0], st6[:, :, 1], st6[:, :, 1])
                nc.gpsimd.tensor_mul(st6[:, :, 3], st6[:, :, 4], st6[:, :, 4])
                nc.gpsimd.tensor_add(st6[:, :, 0], st6[:, :, 0], st6[:, :, 3])
                nc.gpsimd.tensor_add(st6[:, :, 2], st6[:, :, 2], st6[:, :, 5])
                nc.gpsimd.tensor_mul(st6[:, :, 0], st6[:, :, 0],
                                     c256[:].to_broadcast((128, 8)))
                nc.gpsimd.tensor_add(st2[:, :, 1], st6[:, :, 0], st6[:, :, 2])
                # pre-merge the two 512-halves -> [128, ci, 2]
                st2m = fr.tile([128, NCH, 2], F32, tag="st2m")
                st2v = st2[:].rearrange("p (a b) c -> p a b c", b=2)
                nc.gpsimd.tensor_add(st2m[:], st2v[:, :, 0, :], st2v[:, :, 1, :])
                st2_tiles[f] = st2m

            def emit_finish_fold(f):
                # fold over partitions+halves: 2 accumulating matmuls with
                # strided rhs -> psum_g[band j, (ci, kind)] (gmat scaled 1/64)
                ps_g = psT.tile([8, NCH, 2], F32, tag="pst")
                nc.tensor.matmul(
                    ps_g[:], lhsT=prm[:, 8:16], rhs=st2_tiles[f][:],
                    start=True, stop=True)
                gsb = fr.tile([8, NCH, 2], F32, tag="gsb")
                nc.scalar.activation(out=gsb[:], in_=ps_g[:], func=Copy)
                return gsb

            def emit_finish_hx(gsb):
                # hx = 0.5*(var + eps) = gsb1*(0.5/256) - 0.5*mu^2 + 0.5*eps
                msq = fr.tile([8, NCH], F32, tag="msq")
                nc.gpsimd.tensor_mul(msq[:], gsb[:, :, 0], gsb[:, :, 0])
                nc.gpsimd.tensor_mul(msq[:], msq[:],
                                     chalf[:].to_broadcast((8, NCH)))
                hx = fr.tile([8, NCH], F32, tag="hx")
                nc.gpsimd.tensor_mul(hx[:], gsb[:, :, 1],
                                     ci256[:].to_broadcast((8, NCH)))
                nc.gpsimd.tensor_sub(hx[:], hx[:], msq[:])
                nc.gpsimd.tensor_add(hx[:], hx[:],
                                     cepsh[:].to_broadcast((8, NCH)))
                return hx

            def emit_finish_quake(gsb, hx):
                # quake rsqrt, one positive-form Newton step (DVE)
                sh = fr.tile([8, NCH], I32, tag="sh")
                nc.vector.tensor_scalar(
                    out=sh[:], in0=hx[:].bitcast(I32), scalar1=1, scalar2=None,
                    op0=Alu.arith_shift_right)
                ya = fr.tile([8, NCH], F32, tag="ya")
                nc.vector.tensor_sub(ya[:].bitcast(I32), magic_sb[:], sh[:])
                u = fr.tile([8, NCH], F32, tag="u")
                nc.vector.tensor_mul(u[:], ya[:], ya[:])
                nc.vector.tensor_mul(u[:], u[:], hx[:])
                nc.vector.tensor_mul(u[:], u[:], ya[:])
                nc.vector.scalar_tensor_tensor(
                    out=gsb[:, :, 1], in0=ya[:], scalar=1.5, in1=u[:],
                    op0=Alu.mult, op1=Alu.subtract)   # istd = 1.5*ya - ya*u

            def emit_finish_expand(gsb):
                # expand to channels: psum_e[c, (ci, 2)] = emat^T @ gsb
                ps_e = psT.tile([128, NCH, 2], F32, tag="pst")
                nc.tensor.matmul(
                    ps_e[:].rearrange("p a b -> p (a b)"), lhsT=emat_sb[:],
                    rhs=gsb[:].rearrange("p a b -> p (a b)"),
                    start=True, stop=True)
                mi = fr.tile([128, NCH, 2], F32, tag="mi")
                nc.scalar.activation(out=mi[:], in_=ps_e[:], func=Copy)
                return mi

            # ---------------- context constants: k/v, kq, vo -----------------
            kT_f8 = wp.tile([128, NCH, S], FP8)
            vT_sb = wp.tile([128, NCH, S], BF16)

            emit_stats_bn(0)

            for half in range(2):
                wsrc = wkvk_f8 if half == 0 else wkvv_f8
                ps_kv = psT.tile([S, C], F32, tag="pst")
                for i in range(NDCH // 2):
                    nc.tensor.matmul(
                        ps_kv[:], lhsT=ctx_f8[:, 2 * i:2 * i + 2, :],
                        rhs=wsrc[:, 2 * i:2 * i + 2, :],
                        start=(i == 0),
                        stop=(i == NDCH // 2 - 1 and not with_bkv),
                        perf_mode=DR)
                if with_bkv:
                    nc.tensor.matmul(
                        ps_kv[:], lhsT=ones1s[:],
                        rhs=bkv_bf[:, half * C:(half + 1) * C],
                        start=False, stop=True)
                kv_sb = sm.tile([S, C], BF16, tag="kv")
                nc.scalar.activation(out=kv_sb[:], in_=ps_kv[:], func=Copy,
                                     scale=1.0 / W8SCALE)
                ps_t = psT.tile([128, NCH, S], BF16, tag="pst")
                for ci in range(NCH):
                    nc.tensor.transpose(
                        ps_t[:, ci, :], kv_sb[:, ci * 128:(ci + 1) * 128],
                        identity[:64, :64])
                dst = kT_f8 if half == 0 else vT_sb
                nc.scalar.activation(out=dst[:], in_=ps_t[:], func=Copy)


            # kq[c, s] = sum_o wq[o, c] k[s, o]  (f32 kept for per-frame scale)
            kq_sb = wp.tile([128, NCH, S], F32)
            ps_kq = psT.tile([128, NCH, S], F32, tag="pst")
            for co in range(NCH):
                for i in range(NCH // 2):
                    nc.tensor.matmul(
                        ps_kq[:, co, :],
                        lhsT=wq_f8[:, 2 * i:2 * i + 2, co * 128:(co + 1) * 128],
                        rhs=kT_f8[:, 2 * i:2 * i + 2, :],
                        start=(i == 0), stop=(i == NCH // 2 - 1),
                        perf_mode=DR)
            nc.scalar.activation(out=kq_sb[:], in_=ps_kq[:], func=Copy,
                                 scale=1.0 / W8SCALE)
            kq_bf = wp.tile([128, NCH, S], BF16)
            nc.gpsimd.tensor_copy(out=kq_bf[:], in_=kq_sb[:])

            # vo[s, oc] = sum_c v[s, c] wo[oc, c]  (+ bo row: softmax sums to 1)
            vo_bf = wp.tile([S, C], BF16)
            ps_vo = psT.tile([S, C], F32, tag="pst")
            for ci in range(NCH):
                nc.tensor.matmul(
                    ps_vo[:], lhsT=vT_sb[:, ci, :], rhs=wo_bf[:, ci, :],
                    start=(ci == 0), stop=(ci == NCH - 1 and not with_bo))
            if with_bo:
                nc.tensor.matmul(
                    ps_vo[:], lhsT=ones1s[:], rhs=bo_bf[:],
                    start=False, stop=True)
            nc.scalar.activation(out=vo_bf[:], in_=ps_vo[:], func=Copy)

            # bqk[s] = sum_o bq[o] k[s, o] -> folded into all mask columns
            if with_bq:
                bq_bf = wp.tile([128, NCH], FP8)
                nc.gpsimd.tensor_copy(out=bq_bf[:], in_=bqT_sb[:])
                ps_bq = psT.tile([S, 1], F32, tag="pst")
                for ci in range(NCH):
                    nc.tensor.matmul(
                        ps_bq[:], lhsT=kT_f8[:, ci, :], rhs=bq_bf[:, ci:ci + 1],
                        start=(ci == 0), stop=(ci == NCH - 1))
                nc.vector.scalar_tensor_tensor(
                    out=prm[:S, 16:20], in0=ps_bq[:].to_broadcast((S, 4)),
                    scalar=SCALE, in1=prm[:S, 16:20],
                    op0=Alu.mult, op1=Alu.add)

            # ---------------- 2-deep pipelined frame loop --------------------
            # Per-engine FIFO orders are chosen so no engine head-blocks:
            #   DVE : quake(f), bn(f+1), evac-oc2/3(f-1), linv(f)
            #   GPS : merge(f), hx(f), ab/kqf(f), pn(f)
            #   PE  : fold(f), out(f-1) oc0/1, expand(f), bias(f), scores(f),
            #         out oc2, l(f), out oc3
            #   ACT : gsb(f), mi(f), biascol(f), evac-oc0/1(f-1), Exp(f)
            pending = [None]

            def emit_out_mms(ent, oc, preadd):
                bf_, bpn, bx = ent
                ps_o = psO.tile([128, 2, 512], F32, tag="ps_o")
                for hf in range(2):
                    nc.tensor.matmul(
                        ps_o[:, hf, :],
                        lhsT=vo_bf[:, oc * 128:(oc + 1) * 128],
                        rhs=bpn[:, hf, :], start=True, stop=not preadd)
                    if preadd:
                        nc.tensor.matmul(
                            ps_o[:, hf, :], lhsT=identity[:],
                            rhs=bx[:, oc, hf * 512:(hf + 1) * 512],
                            start=False, stop=True)
                return ps_o

            for f in range(FPC):
                x_sb = x_tiles[f]
                ps_sc = psA.tile([S, 2, 512], F32, tag="ps_sc")
                ent = pending[0]
                pending[0] = None

                emit_stats_merge(f, st6_tiles[f])
                gsb = emit_finish_fold(f)

                ps_o01 = []
                if ent is not None:
                    ps_o01.append(emit_out_mms(ent, 0, preadd=True))
                    ps_o01.append(emit_out_mms(ent, 1, preadd=True))

                hx = emit_finish_hx(gsb)
                emit_finish_quake(gsb, hx)
                mi = emit_finish_expand(gsb)

                # a = istd*gamma ; b = beta - mu*a ; kqf = a .* kq (GpSimd)
                ab = fr.tile([128, NCH, 2], F32, tag="ab")
                nc.gpsimd.tensor_mul(ab[:, :, 0], mi[:, :, 1], prm[:, 0:4])
                nc.gpsimd.tensor_mul(ab[:, :, 1], mi[:, :, 0], ab[:, :, 0])
                nc.gpsimd.tensor_sub(ab[:, :, 1], prm[:, 4:8], ab[:, :, 1])
                kqf = fr.tile([128, NCH, S], BF16, tag="kqf")
                nc.gpsimd.tensor_mul(
                    kqf[:], kq_sb[:],
                    ab[:, :, 0:1].to_broadcast((128, NCH, S)))

                b_bf = fr.tile([128, NCH, 1], BF16, tag="b_bf")
                nc.gpsimd.tensor_copy(out=b_bf[:], in_=ab[:, :, 1:2])
                ps_b = psT.tile([S, 1], F32, tag="pst")
                for ci in range(NCH):
                    nc.tensor.matmul(
                        ps_b[:], lhsT=kq_bf[:, ci, :], rhs=b_bf[:, ci, :],
                        start=(ci == 0), stop=(ci == NCH - 1))
                biascol = fr.tile([S, 1], F32, tag="biascol")
                nc.scalar.activation(
                    out=biascol[:], in_=ps_b[:], func=Identity,
                    bias=prm[:S, 16 + f:17 + f], scale=SCALE)

                # ACT evacs of f-1 oc0/1 fill the gap before Exp(f)
                if ent is not None:
                    bx = ent[2]
                    for oc in range(2):
                        nc.scalar.activation(
                            out=bx[:, oc, :],
                            in_=ps_o01[oc][:].rearrange("p a b -> p (a b)"),
                            func=Copy)

                # scoresT[s, q]; p = exp(SCALE*scores + bias)
                for hf in range(2):
                    for ci in range(NCH):
                        nc.tensor.matmul(
                            ps_sc[:, hf, :], lhsT=kqf[:, ci, :],
                            rhs=x_sb[:, ci, hf * 512:(hf + 1) * 512],
                            start=(ci == 0), stop=(ci == NCH - 1))
                p_bf = fr.tile([S, 2, 512], BF16, tag="p_bf")
                nc.scalar.activation(
                    out=p_bf[:], in_=ps_sc[:], func=Exp,
                    bias=biascol[:], scale=SCALE)

                # out(f-1) oc2 | l(f) | out(f-1) oc3 on the PE
                ps_o23 = []
                if ent is not None:
                    ps_o23.append(emit_out_mms(ent, 2, preadd=False))
                for hf in range(2):
                    nc.tensor.matmul(
                        ps_sc[:, hf, :], lhsT=ones64[:], rhs=p_bf[:, hf, :],
                        start=True, stop=True)
                if ent is not None:
                    ps_o23.append(emit_out_mms(ent, 3, preadd=False))

                # next frame's bn_stats ahead of the DVE evacs + linv
                if f + 1 < FPC:
                    emit_stats_bn(f + 1)

                if ent is not None:
                    bf_, bpn, bx = ent
                    for i, oc in enumerate((2, 3)):
                        nc.vector.tensor_tensor(
                            out=bx[:, oc, :],
                            in0=ps_o23[i][:].rearrange("p a b -> p (a b)"),
                            in1=bx[:, oc, :], op=Alu.add)
                    nc.scalar.dma_start(out=out_d[:, bf_, :, :], in_=bx[:])

                linv = fr.tile([S, 2, 512], F32, tag="linv")
                nc.vector.reciprocal_approx_fast(out=linv[:], in_=ps_sc[:])
                pn_bf = fr.tile([S, 2, 512], BF16, tag="pn_bf")
                nc.gpsimd.tensor_mul(pn_bf[:], p_bf[:], linv[:])

                pending[0] = (f, pn_bf, x_sb)

            # final frame flush: ACT evac + per-chunk DMA for earliest drain
            bf_, bpn, bx = pending[0]
            for oc in range(NCH):
                ps_o = emit_out_mms(pending[0], oc, preadd=True)
                nc.scalar.activation(
                    out=bx[:, oc, :],
                    in_=ps_o[:].rearrange("p a b -> p (a b)"), func=Copy)
                nc.scalar.dma_start(out=out_d[:, bf_, oc:oc + 1, :],
                                    in_=bx[:, oc:oc + 1, :])

    nc.finalize()
    return nc


def _prep_in_maps(x, context, gamma, beta, wq, bq, wkv, bkv, wo, bo):
    f32 = lambda a: np.asarray(a, dtype=np.float32)
    bf16c = lambda a: np.ascontiguousarray(a).astype(NP_BF16)
    fp8c = lambda a: np.ascontiguousarray(a).astype(NP_FP8)
    pm = lambda a, n: a.reshape(n, 128, a.shape[-1]).transpose(1, 0, 2)

    wq_c = fp8c(pm(f32(wq) * W8SCALE, NCH))               # [128, 4, C]
    wkvT = f32(wkv).T * W8SCALE                           # [D, 2C]
    wkvk_c = fp8c(pm(np.ascontiguousarray(wkvT[:, :C]), NDCH))
    wkvv_c = fp8c(pm(np.ascontiguousarray(wkvT[:, C:]), NDCH))
    woT_c = bf16c(pm(np.ascontiguousarray(f32(wo).T), NCH))

    prm_base = np.zeros((128, PRM_W), np.float32)
    prm_base[:, 0:4] = f32(gamma).reshape(NCH, 128).T
    prm_base[:, 4:8] = f32(beta).reshape(NCH, 128).T
    pidx = np.arange(128)
    prm_base[pidx, 8 + pidx // CPG] = 1.0 / 64.0

    emat = np.zeros((8, 128), np.float32)
    emat[pidx // CPG, pidx] = 1.0

    bqT_c = np.ascontiguousarray(f32(bq).reshape(NCH, 128).T)
    # kv PSUM carries W8SCALE*k (fp8 weight pre-scale); bias must match
    bkv_c = np.ascontiguousarray(f32(bkv).reshape(1, 2 * C)) * W8SCALE
    bo_r = np.ascontiguousarray(f32(bo).reshape(1, C))

    x_f = f32(x)
    ctx_f = f32(context)

    in_maps = []
    for core in range(NCORES):
        b, r = divmod(core, 4)
        xs = bf16c(
            x_f[b, :, r::4, :, :].reshape(NCH, 128, FPC, HW).transpose(1, 2, 0, 3))
        ctxT = fp8c(pm(np.ascontiguousarray(ctx_f[b].T), NDCH))  # [128, 8, S]
        prm = prm_base.copy()
        for f in range(FPC):
            t = 4 * f + r
            lim = min(4 * (t + 1), S)
            prm[lim:S, 16 + f] = NEGINF
        m = dict(x=xs, ctxT_pm=ctxT, wq_pm=wq_c, wkvk_pm=wkvk_c,
                 wkvv_pm=wkvv_c, wo_pm=woT_c, prm=prm, emat=emat)
        if np.any(bqT_c):
            m["bqT"] = bqT_c
        if np.any(bkv_c):
            m["bkv"] = bkv_c
        if np.any(bo_r):
            m["bo"] = bo_r
        in_maps.append(m)
    return in_maps


def kernel(x, context, gamma, beta, wq, bq, wkv, bkv, wo, bo,
           _trace=False, **_trace_kwargs):
    global LAST_RESULT
    with_bq = bool(np.any(np.asarray(bq)))
    with_bkv = bool(np.any(np.asarray(bkv)))
    with_bo = bool(np.any(np.asarray(bo)))
    key = (with_bq, with_bkv, with_bo)
    if key not in _GRAPH_CACHE:
        _GRAPH_CACHE[key] = _build(*key)
    nc = _GRAPH_CACHE[key]

    in_maps = _prep_in_maps(x, context, gamma, beta, wq, bq, wkv, bkv, wo, bo)
    res = run_bass_kernel_spmd(nc, in_maps, core_ids=list(range(NCORES)),
                               trace=_trace, **_trace_kwargs)
    LAST_RESULT = res

    out = np.empty((B, C, T, H, W), np.float32)
    for core in range(NCORES):
        b, r = divmod(core, 4)
        arr = np.asarray(res.results[core]["out"], dtype=np.float32)
        out[b, :, r::4, :, :] = arr.transpose(2, 0, 1, 3).reshape(C, FPC, H, W)
    return out
